# revision 56
# baseline (speedup 1.0000x reference)
"""Multi-head attention (B=2, T=2048, D=1024, H=16) on 8 NeuronCores.

Tensor-parallel over heads: 2 heads per core. Each core computes its
heads' QKV projection, causal attention, and a partial output
projection (its 128 columns of the concat dim); partials are summed on
the host.

Device dataflow is fully "transposed" (feature-major):
  - host supplies x^T as [128, 8, 4096] (partition, ktile, token)
  - qkv^T = W_slice @ x^T        (per-core W rows, pre-transposed host-side)
  - S^T[k,q] block = matmul(lhsT=K^T tile, rhs=Q^T tile), contraction dh=64
  - P^T = exp(S^T/8); only the 128-wide diagonal strips need causal
    masking (triangle-mask multiply on DVE); the j=1,2,3 diagonal
    blocks are packed into one PSUM tile for a single batched exp
  - O_aug^T [65, q] = V_aug.T @ P^T  with V_aug = [V | 1] so row 64
    accumulates the softmax denominator for free
  - normalize: f16 DVE reciprocal of the denominator row, fanned out
    across partitions with two DVE stream_shuffle crossbar passes
    (gpsimd partition_broadcast is a Q7 software op and is ~100x
    slower on HW than the cost model suggests)
  - y^T partial [D, B*T] = (W_out slice)^T.T @ concatO^T

Scheduling: the whole kernel is emitted as one software pipeline.
Attention units are issued two stages deep — each push emits S(u),
exp(u-1), O(u-2) — so every exp has a full unit of PE work in front
of its O matmuls and the PE never waits on a fresh exp; projection
(phase A) and output projection (phase C) chunks are woven between
attention units so the Activation engine (exp) and PE stay
concurrently busy across the whole kernel. Both heads' V tiles are
transposed in a single PE transpose and scatter-copied into the
augmented V tiles.
"""

import sys

sys.path.insert(0, "/opt/trn_rl_repo")

import numpy as np

import concourse.bass as bass
import concourse.mybir as mybir
import concourse.tile as tile
from concourse import bacc
from concourse.masks import make_identity

B = 2
T = 2048
D = 1024
H = 16
DH = 64
N_CORES = 8
HPC = H // N_CORES          # heads per core = 2
F = HPC * DH                # per-core feature block = 128
TOK = B * T                 # 4096
P = 128                     # partitions
QB = 512                    # q block (free dim of S^T tiles)
KB = 128                    # k block (partition dim of S^T tiles)
NQB = T // QB               # 4 q blocks per batch
NKB = T // KB               # 16 k blocks per batch
NTT = TOK // QB             # 8 token tiles for projections
NKT = D // P                # 8 contraction tiles over D

F32 = mybir.dt.float32
F32R = mybir.dt.float32r
F16 = mybir.dt.float16
EXP = mybir.ActivationFunctionType.Exp


class BPipe:
    """Two-stage software pipeline for S->exp->O units: each push emits
    S(u), exp(u-1), O(u-2), so the exp has a full unit of PE work in
    front of its O and the O matmuls never wait on a fresh exp. PSUM
    still fits: S(u+2) reuses S(u)'s buffer, whose consumer exp(u) is
    already emitted one push earlier."""

    def __init__(self):
        self.pend_e = None
        self.pend_o = None

    def push(self, s_emit, e_emit, o_emit):
        s_emit()
        fired_e_o = None
        if self.pend_e is not None:
            e, o = self.pend_e
            e()
            fired_e_o = o
        if self.pend_o is not None:
            self.pend_o()
        self.pend_o = fired_e_o
        self.pend_e = (e_emit, o_emit)

    def flush(self):
        if self.pend_e is not None:
            e, o = self.pend_e
            e()
            if self.pend_o is not None:
                self.pend_o()
            o()
            self.pend_e = None
            self.pend_o = None
        elif self.pend_o is not None:
            self.pend_o()
            self.pend_o = None


def build_nc(loop_n: int = 1, phases: str = "ABC"):
    nc = bacc.Bacc()

    xT = nc.dram_tensor("xT", [P, NKT, TOK], F16, kind="ExternalInput")
    wqkvT = nc.dram_tensor("wqkvT", [P, NKT, 3 * F], F16, kind="ExternalInput")
    woT = nc.dram_tensor("woT", [F, D], F32R, kind="ExternalInput")
    yT = nc.dram_tensor("yT", [D, TOK], F16, kind="ExternalOutput")

    with tile.TileContext(nc) as tc:
        with (
            tc.tile_pool(name="const", bufs=1) as const,
            tc.tile_pool(name="big", bufs=1) as big,
            tc.tile_pool(name="xin", bufs=3) as xin,
            tc.tile_pool(name="psb", bufs=8) as psb,
            tc.tile_pool(name="smr", bufs=2) as smr,
            tc.tile_pool(name="smrr", bufs=2) as smrr,
            # qb-major phase C keeps all 8 per-batch y tiles alive
            tc.tile_pool(name="ysb", bufs=10) as ysb,
            tc.tile_pool(name="psA", bufs=2, space="PSUM") as psA,
            tc.tile_pool(name="psB", bufs=2, space="PSUM") as psB,
            tc.tile_pool(name="psO", bufs=2, space="PSUM") as psO,
        ):
            import contextlib

            loop_ctx = (
                tc.For_i(0, loop_n, 1) if loop_n > 1 else contextlib.nullcontext()
            )
            with loop_ctx:
                build_body(nc, tc, const, big, xin, psb, smr, smrr, ysb,
                           psA, psB, psO, xT, wqkvT, woT, yT, phases)

    nc.compile()
    return nc


def build_body(nc, tc, const, big, xin, psb, smr, smrr, ysb,
               psA, psB, psO, xT, wqkvT, woT, yT, phases="ABC"):
    xts = {}

    def emit_xt_load(b, tpl, engine, nchunk=4):
        # one token-pair tile [128, 8, 1024], split into nchunk DMA
        # triggers so transfers spread across DMA engines
        xt = xin.tile([P, NKT, 2 * QB], F16, name=f"xt_{b}_{tpl}", tag="xt")
        c0 = b * T + tpl * 2 * QB
        step = NKT // nchunk
        for q in range(nchunk):
            engine.dma_start(
                out=xt[:, step * q : step * (q + 1), :],
                in_=xT[:, step * q : step * (q + 1), c0 : c0 + 2 * QB],
            )
        xts[(b, tpl)] = xt

    # ---- input loads first: per-kt weight chunks on SP, the first
    # token tile per-kt on ACT (fast HWDGE) + Pool, so the first
    # projection matmul starts ~1.5us in and DMA saturates from t=0
    w_sb = const.tile([P, NKT, 3 * F], F16, tag="w_sb")
    for kt in range(NKT):
        nc.sync.dma_start(out=w_sb[:, kt, :], in_=wqkvT[:, kt, :])
    xt00 = xin.tile([P, NKT, 2 * QB], F16, name="xt_0_0", tag="xt")
    nc.scalar.dma_start(out=xt00[:, 0, 0:QB], in_=xT[:, 0, 0:QB])
    nc.scalar.dma_start(out=xt00[:, 0, QB : 2 * QB], in_=xT[:, 0, QB : 2 * QB])
    for kt in range(1, NKT):
        eng = nc.scalar if kt < 4 else nc.gpsimd
        eng.dma_start(
            out=xt00[:, kt, :], in_=xT[:, kt, 0 : 2 * QB]
        )
    xts[(0, 0)] = xt00
    emit_xt_load(0, 1, nc.sync)
    wo_sb = const.tile([P, D], F32R, tag="wo_sb")
    nc.sync.dma_start(out=wo_sb[:], in_=woT[:, :])

    # ---- constants ----
    # identity built in f32 (walrus rejects memset on f32r), used as f32r
    ident32 = const.tile([P, P], F32, tag="ident32")
    make_identity(nc, ident32[:])
    ident = const.tile([P, P], F32R, tag="ident")
    nc.vector.tensor_copy(ident[:], ident32[:])
    # tri[krow, t] = 1.0 iff t >= krow — the only causal mask needed:
    # every diagonal 128x128 strip uses the same triangle
    tri32 = const.tile([P, KB], F32, tag="tri32")
    nc.gpsimd.memset(tri32[:], 1.0)
    nc.gpsimd.affine_select(
        out=tri32[:], in_=tri32[:],
        compare_op=mybir.AluOpType.is_ge, fill=0.0,
        base=0, channel_multiplier=-1, pattern=[[1, KB]],
    )
    tri = const.tile([P, KB], F16, tag="tri")
    nc.vector.tensor_copy(tri[:], tri32[:])

    # ---- activation tiles ----
    QTs = [big.tile([P, QB], F32R, tag=f"QT{i}", name=f"QTs{i}") for i in range(NTT)]
    KTs = [big.tile([P, QB], F32R, tag=f"KT{i}", name=f"KTs{i}") for i in range(NTT)]
    # V^T tiles double as CO tiles: V^T is consumed by the V_aug
    # transposes before the CO writes of the same batch begin.
    VCs = [big.tile([P, QB], F32R, tag=f"VC{i}", name=f"VCs{i}") for i in range(NTT)]
    # V_aug per (b, ki): [tok 128, head 2, dh+1 65]; col 64 stays 1.0
    Vaugs = [
        big.tile([P, HPC, DH + 1], F16, tag=f"Va{vi}", name=f"Vaug{vi}")
        for vi in range(B * NKB)
    ]

    def cp(eng, out, in_):
        if eng == "act":
            nc.scalar.copy(out, in_)
        else:
            nc.vector.tensor_copy(out, in_)

    def tri_strip(strip):
        # zero the below-diagonal triangle of a 128x128 P^T strip via a
        # triangle-mask multiply on DVE (lower latency than Pool's
        # affine_select in the exp->mask->O chain)
        nc.vector.tensor_mul(strip, strip, tri[:])

    def emit_A_tp(b, tpl, qk_eng, v_eng):
        tt0 = 2 * (2 * b + tpl)
        xt = xts[(b, tpl)]
        prA = [psA.tile([P, 2 * QB], F32, name=f"prA_{b}_{tpl}_{h}", tag="psA")
               for h in range(2)]
        prB = [psB.tile([P, QB], F32, name=f"prB_{b}_{tpl}_{h}", tag="psB")
               for h in range(2)]
        for kt in range(NKT):
            st, sp = (kt == 0), (kt == NKT - 1)
            for g in range(3):
                lhs = w_sb[:, kt, g * F : (g + 1) * F]
                for half in range(2):
                    rhs = xt[:, kt, half * QB : (half + 1) * QB]
                    if g == 0:
                        dst = prA[half][:, 0:QB]
                    elif g == 1:
                        dst = prA[half][:, QB:]
                    else:
                        dst = prB[half][:]
                    nc.tensor.matmul(dst, lhs, rhs, start=st, stop=sp)
        # interleave V (feeds the V_aug transposes) with Q/K (feeds
        # the S units) so neither consumer starves on copy backlog
        for half in range(2):
            tt = tt0 + half
            cp(v_eng, VCs[tt][:], prB[half][:])
            cp(qk_eng, QTs[tt][:], prA[half][:, 0:QB])
            cp(qk_eng, KTs[tt][:], prA[half][:, QB:])

    def emit_A2(b, kis, va_eng="dve", ms_eng="gpsimd"):
        for ki in kis:
            # b=0's memsets go on DVE: the Pool queue is still busy
            # issuing the startup SWDGE x loads at that point
            if ms_eng == "dve":
                nc.vector.memset(Vaugs[b * NKB + ki][:], 1.0)
            else:
                nc.gpsimd.memset(Vaugs[b * NKB + ki][:], 1.0)
        for ki in kis:
            src = VCs[2 * NQB * b // 2 + ki // 4][:, (ki % 4) * KB : (ki % 4 + 1) * KB]
            tr = psB.tile([P, KB], F32R, name=f"tr_{b}_{ki}", tag="psB")
            nc.tensor.matmul(tr[:], src, ident[:], is_transpose=True,
                             start=True, stop=True)
            cp(va_eng, Vaugs[b * NKB + ki][:, :, 0:DH], tr[:])

    # ---- attention units ----
    o_ps_cur = {}

    def push_qi_units(pipe, b, h, qi, filler=None):
        def push(s_emit, e_emit, o_emit):
            pipe.push(s_emit, e_emit, o_emit)
            if filler is not None:
                filler()

        qrows = np.s_[h * DH : (h + 1) * DH]
        qt = QTs[NQB * b + qi]
        # full-bank tile: rows 0:64 = O, row 64 = denominator, rows
        # 64:128 reused by the reciprocal broadcast (overwrites the
        # dead denominator row)
        o_ps = psO.tile([P, QB], F32, name=f"ops_{b}_{h}_{qi}", tag="o")
        vbase = b * NKB

        def vaug(ki):
            return Vaugs[vbase + ki][:, h, :]

        def kslc(ki):
            return KTs[NQB * b + ki // 4][qrows, (ki % 4) * KB : (ki % 4 + 1) * KB]

        # full pairs
        for k2 in range(2 * qi):
            kis = (2 * k2, 2 * k2 + 1)
            sA = psA.tile([P, 2 * QB], F32, name=f"s_{b}_{h}_{qi}_{k2}", tag="psA")

            def S(sA=sA, kis=kis):
                for idx, ki in enumerate(kis):
                    nc.tensor.matmul(
                        sA[:, idx * QB : (idx + 1) * QB], kslc(ki), qt[qrows, :],
                        start=True, stop=True,
                    )

            p2box = {}

            def E(sA=sA, k2=k2, p2box=p2box):
                p2 = psb.tile([P, 2 * QB], F16, name=f"p_{b}_{h}_{qi}_{k2}", tag="p")
                nc.scalar.activation(p2[:], sA[:], EXP, scale=0.125)
                p2box["t"] = p2

            def O(kis=kis, k2=k2, p2box=p2box):
                p2 = p2box["t"]
                for idx, ki in enumerate(kis):
                    nc.tensor.matmul(
                        o_ps[0 : DH + 1, :], vaug(ki),
                        p2[:, idx * QB : (idx + 1) * QB],
                        start=(k2 == 0 and idx == 0), stop=False,
                    )

            push(S, E, O)

        # diagonal block j=0 (full width, strip mask at cols 0:128).
        # Drawn from psA (not psB) to keep psB free for A/C/transpose use.
        ki0 = 4 * qi

        sB = psA.tile([P, QB], F32, name=f"sd0_{b}_{h}_{qi}", tag="psA")

        def S0(sB=sB, ki0=ki0):
            nc.tensor.matmul(sB[:], kslc(ki0), qt[qrows, :], start=True, stop=True)

        p0box = {}

        def E0(sB=sB, p0box=p0box):
            p2 = psb.tile([P, QB], F16, name=f"pd0_{b}_{h}_{qi}", tag="p")
            nc.scalar.activation(p2[:], sB[:], EXP, scale=0.125)
            tri_strip(p2[:, 0:KB])
            p0box["t"] = p2

        def O0(ki0=ki0, qi=qi, p0box=p0box):
            nc.tensor.matmul(o_ps[0 : DH + 1, :], vaug(ki0), p0box["t"][:],
                             start=(qi == 0), stop=False)

        push(S0, E0, O0)

        # diagonal blocks j=1,2,3 packed into one PSUM tile / one exp.
        # Matmul outputs may not cross a PSUM bank (512-col) boundary:
        # j1 fills bank0 cols 0:384, j3 cols 384:512, j2 bank1 cols 512:768.
        packs = [(1, 0, QB - KB), (3, QB - KB, KB), (2, QB, QB - 2 * KB)]
        sA = psA.tile([P, 2 * QB], F32, name=f"sd_{b}_{h}_{qi}", tag="psA")

        def S123(sA=sA, qi=qi):
            for j, off, w in packs:
                ki = 4 * qi + j
                nc.tensor.matmul(
                    sA[:, off : off + w], kslc(ki), qt[qrows, j * KB : QB],
                    start=True, stop=True,
                )

        pdbox = {}

        def E123(sA=sA, qi=qi, b=b, h=h, pdbox=pdbox):
            width = 2 * QB - 2 * KB  # 768
            p2 = psb.tile([P, width], F16, name=f"pd_{b}_{h}_{qi}", tag="p")
            nc.scalar.activation(p2[:], sA[:, 0:width], EXP, scale=0.125)
            for j, off, w in packs:
                tri_strip(p2[:, off : off + KB])
            pdbox["t"] = p2

        def O123(qi=qi, b=b, h=h, o_ps=o_ps, pdbox=pdbox):
            p2 = pdbox["t"]
            for n, (j, off, w) in enumerate(packs):
                ki = 4 * qi + j
                nc.tensor.matmul(
                    o_ps[0 : DH + 1, j * KB : QB], vaug(ki),
                    p2[:, off : off + w],
                    start=False, stop=(n == len(packs) - 1),
                )
            # normalize: rows 0:64 / row 64 (still inside the O-part,
            # after the accumulation group stops).
            # f16 reciprocal/broadcast: 1/d at f16 costs ~5e-4 relative
            # (budget is 2e-2) and halves the DVE crossbar time
            r = smr.tile([32, QB], F16, name=f"r_{b}_{h}_{qi}", tag="r")
            nc.gpsimd.memset(r[:], 0.0)
            with nc.allow_low_precision(reason="softmax recip f16 bcast"):
                nc.vector.reciprocal(r[0:1, :], o_ps[DH : DH + 1, :])
            rr = smrr.tile([DH, QB], F16, name=f"rr_{b}_{h}_{qi}", tag="rr")
            bmask = [0] * 32
            nc.vector.stream_shuffle(rr[0:32, :], r[0:32, :], bmask)
            nc.vector.stream_shuffle(rr[32:64, :], r[0:32, :], bmask)
            nc.vector.tensor_mul(
                VCs[NQB * b + qi][qrows, :], o_ps[0:DH, :], rr[:]
            )

        push(S123, E123, O123)

    # ---- phase C: y^T partial = woT.T @ CO, emitted in qb-major
    # chunks so each chunk depends only on the CO token-block that the
    # attention pipeline just finished ----
    ysbs = {}

    def C_chunk(b, qb, oi, eng):
        if qb == 0:
            ysbs[(b, oi)] = ysb.tile([P, T], F16, name=f"ysb_{b}_{oi}", tag="y")
        y_sb = ysbs[(b, oi)]
        y2 = psB.tile([P, QB], F32, name=f"y2_{b}_{oi}_{qb}", tag="psB")
        nc.tensor.matmul(
            y2[:], wo_sb[:, oi * P : (oi + 1) * P], VCs[NQB * b + qb][:],
            start=True, stop=True,
        )
        cp(eng, y_sb[:, qb * QB : (qb + 1) * QB], y2[:])
        if qb == 3:
            nc.sync.dma_start(
                out=yT[oi * P : (oi + 1) * P, b * T : (b + 1) * T], in_=y_sb[:]
            )

    def C_chunk2(b, oi, q2, eng):
        if q2 == 0:
            ysbs[(b, oi)] = ysb.tile([P, T], F16, name=f"ysb_{b}_{oi}", tag="y")
        y_sb = ysbs[(b, oi)]
        for qb in (2 * q2, 2 * q2 + 1):
            y2 = psB.tile([P, QB], F32, name=f"y2_{b}_{oi}_{qb}", tag="psB")
            nc.tensor.matmul(
                y2[:], wo_sb[:, oi * P : (oi + 1) * P], VCs[NQB * b + qb][:],
                start=True, stop=True,
            )
            cp(eng, y_sb[:, qb * QB : (qb + 1) * QB], y2[:])
        if q2 == 1:
            nc.sync.dma_start(
                out=yT[oi * P : (oi + 1) * P, b * T : (b + 1) * T], in_=y_sb[:]
            )

    import collections as _c

    fill = _c.deque()

    def filler():
        if fill:
            fill.popleft()()

    def queue_C(b, qb, eng="dve"):
        for oi in range(D // P):
            fill.append(lambda b=b, qb=qb, oi=oi: C_chunk(b, qb, oi, eng))

    # ================= emission =================
    pipe = BPipe()

    emit_A_tp(0, 0, "act", "dve")
    emit_A2(0, range(8), ms_eng="dve")

    if phases == "A":
        emit_A_tp(0, 1, "dve", "dve")
        emit_A2(0, range(8, 16))
        emit_xt_load(1, 0, nc.sync)
        emit_A_tp(1, 0, "dve", "dve")
        emit_A2(1, range(8))
        emit_xt_load(1, 1, nc.sync)
        emit_A_tp(1, 1, "dve", "dve")
        emit_A2(1, range(8, 16))
        for tt in range(NTT):
            for k, ts_ in enumerate((QTs, KTs, VCs)):
                nc.gpsimd.dma_start(
                    out=yT[k * P : (k + 1) * P, tt : tt + 1], in_=ts_[tt][:, 0:1]
                )
        for vi in range(B * NKB):
            nc.gpsimd.dma_start(
                out=yT[3 * P : 3 * P + P, vi : vi + 1], in_=Vaugs[vi][:, 0, 0:1]
            )
        return

    # b=0 attention; b=1 x loads issued from inside the stream
    for h in range(HPC):
        for qi in (0, 1):
            push_qi_units(pipe, 0, h, qi)
            if h == 0 and qi == 0:
                emit_xt_load(1, 0, nc.sync)
    emit_A_tp(0, 1, "dve", "dve")
    emit_A2(0, range(8, 16))
    for h in range(HPC):
        for qi in (2, 3):
            push_qi_units(pipe, 0, h, qi)
            if h == 0 and qi == 2:
                emit_xt_load(1, 1, nc.sync)

    if phases == "AB":
        pipe.flush()
        for i in range(NQB):
            nc.gpsimd.dma_start(out=yT[0:P, i * QB : (i + 1) * QB], in_=VCs[i][:])
        return

    emit_A_tp(1, 0, "act", "dve")
    emit_A2(1, range(8))

    # b=1 attention with b=0 out-projection woven in at (oi, qb-pair)
    # granularity (two matmuls sharing the stationary + two copies)
    c0_chunks = [(oi, q2) for oi in range(D // P) for q2 in range(2)]
    ci = 0

    def pop_C0(eng="dve"):
        nonlocal ci
        if ci < len(c0_chunks):
            oi, q2 = c0_chunks[ci]
            C_chunk2(0, oi, q2, eng)
            ci += 1

    b1_qi01 = [(h, qi) for h in range(HPC) for qi in (0, 1)]
    for idx, (h, qi) in enumerate(b1_qi01):
        push_qi_units(pipe, 1, h, qi, filler=(pop_C0 if idx >= 1 else None))
    emit_A_tp(1, 1, "dve", "dve")
    emit_A2(1, range(8, 16))
    b1_qi23 = [(h, qi) for h in range(HPC) for qi in (2, 3)]
    for idx, (h, qi) in enumerate(b1_qi23):
        push_qi_units(pipe, 1, h, qi, filler=pop_C0)
    pipe.flush()
    while ci < len(c0_chunks):
        pop_C0()
    # b=1 out-projection tail: alternate oi groups between psA wide
    # draws and (now-idle) psO narrow draws for 4-deep PSUM rotation;
    # copies split across ACT and DVE
    for oi in range(D // P):
        y_sb = ysb.tile([P, T], F16, name=f"ysb_1_{oi}", tag="y")
        if oi % 2 == 0:
            for q2 in range(NQB // 2):
                y2 = psA.tile([P, 2 * QB], F32, name=f"y2w_{oi}_{q2}",
                              tag="psA")
                for half in range(2):
                    qb = 2 * q2 + half
                    nc.tensor.matmul(
                        y2[:, half * QB : (half + 1) * QB],
                        wo_sb[:, oi * P : (oi + 1) * P], VCs[NQB + qb][:],
                        start=True, stop=True,
                    )
                cp("act", y_sb[:, q2 * 2 * QB : q2 * 2 * QB + QB], y2[:, 0:QB])
                cp("dve", y_sb[:, q2 * 2 * QB + QB : (q2 + 1) * 2 * QB],
                   y2[:, QB:])
        else:
            for qb in range(NQB):
                # alternate psO/psB (both idle at the tail) for 4-deep
                # narrow rotation
                pool = psO if qb % 2 == 0 else psB
                y2 = pool.tile([P, QB], F32, name=f"y2o_{oi}_{qb}",
                               tag="o" if qb % 2 == 0 else "psB")
                nc.tensor.matmul(
                    y2[:], wo_sb[:, oi * P : (oi + 1) * P], VCs[NQB + qb][:],
                    start=True, stop=True,
                )
                cp("act" if qb % 2 == 0 else "dve",
                   y_sb[:, qb * QB : (qb + 1) * QB], y2[:])
        nc.sync.dma_start(
            out=yT[oi * P : (oi + 1) * P, T : 2 * T], in_=y_sb[:]
        )


_NC = None


def get_nc():
    global _NC
    if _NC is None:
        _NC = build_nc()
    return _NC


def make_core_inputs(x, W_in, W_out):
    """Host-side sharding: per-core input maps."""
    xT2 = np.ascontiguousarray(x.reshape(TOK, D).T).astype(np.float16)
    xTh = np.ascontiguousarray(xT2.reshape(NKT, P, TOK).transpose(1, 0, 2))
    in_maps = []
    for c in range(N_CORES):
        rows = np.concatenate(
            [W_in[i * D + c * F : i * D + (c + 1) * F] for i in range(3)], axis=0
        )  # [384, 1024] = q|k|v rows for this core's 2 heads
        w2 = rows.T.astype(np.float16)  # [1024, 384]
        wqkvTh = np.ascontiguousarray(w2.reshape(NKT, P, 3 * F).transpose(1, 0, 2))
        woTh = np.ascontiguousarray(W_out[:, c * F : (c + 1) * F].T).astype(
            np.float32
        )
        in_maps.append({"xT": xTh, "wqkvT": wqkvTh, "woT": woTh})
    return in_maps


def kernel(x, W_in, W_out):
    from concourse.bass_utils import run_bass_kernel_spmd

    nc = get_nc()
    in_maps = make_core_inputs(
        np.asarray(x, dtype=np.float32),
        np.asarray(W_in, dtype=np.float32),
        np.asarray(W_out, dtype=np.float32),
    )
    res = run_bass_kernel_spmd(nc, in_maps, list(range(N_CORES)))
    y = np.zeros((D, TOK), dtype=np.float32)
    for r in res.results:
        y += r["yT"].astype(np.float32)
    return np.ascontiguousarray(y.T).reshape(B, T, D)
